# revision 31
# baseline (speedup 1.0000x reference)
"""Multi-head attention (B=4, S=2048, D=1024, H=16) on 8 TRN2 NeuronCores.

Sharding: 2D grid (batch x head-group). Core c = g*4 + b handles batch b and
head group g (8 heads = 512 of the 1024 embedding columns).

v4 design: all matmul operands bf16. V computed directly in [keys, dh]
layout (one N=512 matmul group per 128-key tile covering all 4 head pairs).
Scores for a pair's two heads issue as adjacent row-tiled matmuls
(rows 0-63 / 64-127, concurrent on the PE). ScalarE does ONLY the exp (one
[128,1024] activation per (pair, qt, kt)) — it is the bottleneck engine, so
everything else is kept off it. All PSUM evictions on the DVE. V/O biases
are folded into the host combine (softmax rows sum to 1, so
attn(V + bv) = attn(V) + bv exactly).

Scheduling: the key-tile loop of each (pair, 512-query block) is
exp-bound; its spare PE/DVE slots absorb, via a work queue consumed one
item per key tile: the previous block's softmax normalization (whose
denominator partition-broadcast rides a DMA round trip — deferring it
keeps the DVE's in-order queue from blocking PSUM evictions), the next
pair's Q/K projection groups, the V tiles (first block only), and the
out-projection of the previous query block (last pair only). x^T is DMA'd
in 512-column slices so the first projections start as soon as ~1MB has
landed. Host sums the two head-group partials and adds bo + bv @ Wo.
"""
import numpy as np

B, S, D, H, DH = 4, 2048, 1024, 16, 64
NCORES = 8
GCOLS = D // 2          # 512 cols per head-group core
NPAIRS = GCOLS // 128   # 4 head-pairs per core
NKT = S // 128          # 16 key tiles
NQT = S // 512          # 4 query blocks of 512
DC = D // 128           # 8 contraction chunks for projections

_COMPILED = None


def _build():
    import concourse.bass as bass
    import concourse.bacc as bacc
    import concourse.tile as tile
    from concourse import mybir
    from contextlib import ExitStack

    F32 = mybir.dt.float32
    BF16 = mybir.dt.bfloat16
    EXP = mybir.ActivationFunctionType.Exp

    nc = bacc.Bacc("TRN2", target_bir_lowering=False, debug=False)
    xT = nc.dram_tensor("xT", [128, DC, S], BF16, kind="ExternalInput").ap()
    wq = nc.dram_tensor("wq", [128, DC, GCOLS], BF16,
                        kind="ExternalInput").ap()
    wk = nc.dram_tensor("wk", [128, DC, GCOLS], BF16,
                        kind="ExternalInput").ap()
    wv = nc.dram_tensor("wv", [128, DC, GCOLS], BF16,
                        kind="ExternalInput").ap()
    wo = nc.dram_tensor("wo", [128, NPAIRS, D], BF16,
                        kind="ExternalInput").ap()
    bq = nc.dram_tensor("bq", [128, NPAIRS], F32, kind="ExternalInput").ap()
    bk = nc.dram_tensor("bk", [128, NPAIRS], F32, kind="ExternalInput").ap()
    out = nc.dram_tensor("out", [S, D], BF16,
                         kind="ExternalOutput").ap()

    with tile.TileContext(nc) as tc, ExitStack() as st:
        const = st.enter_context(tc.tile_pool(name="const", bufs=1))
        persist = st.enter_context(tc.tile_pool(name="persist", bufs=1))
        qkpool = st.enter_context(tc.tile_pool(name="qkpool", bufs=2))
        espool = st.enter_context(tc.tile_pool(name="espool", bufs=6))
        small = st.enter_context(tc.tile_pool(name="small", bufs=4))
        osb = st.enter_context(tc.tile_pool(name="osb", bufs=12))
        pssc = st.enter_context(
            tc.tile_pool(name="pssc", bufs=2, space="PSUM"))
        psav = st.enter_context(
            tc.tile_pool(name="psav", bufs=2, space="PSUM"))
        pspj = st.enter_context(
            tc.tile_pool(name="pspj", bufs=2, space="PSUM"))

        # --- warmup constants first so the DVE memsets clear quickly
        zw = const.tile([128, 128], BF16)
        zf = const.tile([128, 512], BF16)
        nc.vector.memset(zw, 0.0)
        nc.vector.memset(zf, 0.0)

        # --- biases (per-partition scalars: partition r = within-pair dim,
        # col p = pair index)
        bq_sb = const.tile([128, NPAIRS], F32)
        bk_sb = const.tile([128, NPAIRS], F32)

        # --- resident inputs (bf16). x^T lands in 512-column slices,
        # nt-major, so the first projection groups can start early.
        xT_sb = persist.tile([128, DC, S], BF16, name="xT_sb")
        xT_dram = xT
        wq_sb = persist.tile([128, DC, GCOLS], BF16, name="wq_sb")
        wk_sb = persist.tile([128, DC, GCOLS], BF16, name="wk_sb")
        wv_sb = persist.tile([128, DC, GCOLS], BF16, name="wv_sb")
        wo_sb = persist.tile([128, NPAIRS, D], BF16, name="wo_sb")
        nc.scalar.dma_start(out=wk_sb, in_=wk)
        for dch in range(2):
            dsl = slice(dch * 4, (dch + 1) * 4)
            eng = nc.sync if dch == 0 else nc.gpsimd
            eng.dma_start(out=xT_sb[:, dsl, 0:512], in_=xT_dram[:, dsl, 0:512])
        nc.sync.dma_start(out=bq_sb, in_=bq)
        nc.sync.dma_start(out=bk_sb, in_=bk)
        for dch in range(2):
            dsl = slice(dch * 4, (dch + 1) * 4)
            eng = nc.sync if dch == 0 else nc.gpsimd
            eng.dma_start(out=xT_sb[:, dsl, 512:S],
                          in_=xT_dram[:, dsl, 512:S])
        nc.scalar.dma_start(out=wq_sb, in_=wq)
        nc.gpsimd.dma_start(out=wv_sb, in_=wv)
        nc.scalar.dma_start(out=wo_sb, in_=wo)

        # --- V in [key, dh] layout, ones column at dh=64 per head
        v_sb = persist.tile([128, NKT, 8, 65], BF16, name="v_sb")
        nc.vector.memset(v_sb[:, :, :, 64:65], 1.0)

        # --- attention outputs (transposed), bf16 for the out-projection
        attnT = [persist.tile([128, S], BF16, name=f"attnT{p}",
                              tag=f"attnT{p}") for p in range(NPAIRS)]

        # --- HAM warmup while the input DMAs land
        warm_ps = pspj.tile([128, 512], F32, name="warm_ps", tag="pspj")
        for _ in range(24):
            nc.tensor.matmul(warm_ps, zw, zf, start=True, stop=True,
                             skip_group_check=True)

        qk_tiles = {}

        def proj_group(p, mat, nt):
            """One 512-col block of the Q^T/K^T projection for pair p."""
            for h in proj_halves(p, mat, nt):
                h()

        def proj_halves(p, mat, nt):
            """The projection block as two half-size thunks (sub-us PE
            bursts absorb better into the exp-bound key-tile loop)."""
            if (p, mat) not in qk_tiles:
                qk_tiles[(p, mat)] = qkpool.tile(
                    [128, S], BF16, name=f"{mat}t{p}", tag=f"{mat}t")
            t_sb = qk_tiles[(p, mat)]
            w_sb, b_sb = (wq_sb, bq_sb) if mat == "q" else (wk_sb, bk_sb)
            csl = slice(p * 128, (p + 1) * 128)
            cell = {}

            def first():
                cell["ps"] = pspj.tile([128, 512], F32, name="proj_ps",
                                       tag="pspj")
                for dc in range(DC // 2):
                    nc.tensor.matmul(cell["ps"], w_sb[:, dc, csl],
                                     xT_sb[:, dc, nt * 512:(nt + 1) * 512],
                                     start=(dc == 0), stop=False)

            def second():
                ps = cell["ps"]
                for dc in range(DC // 2, DC):
                    nc.tensor.matmul(ps, w_sb[:, dc, csl],
                                     xT_sb[:, dc, nt * 512:(nt + 1) * 512],
                                     start=False, stop=(dc == DC - 1))
                nc.vector.tensor_scalar_add(
                    t_sb[:, nt * 512:(nt + 1) * 512], ps, b_sb[:, p:p + 1])
            return [first, second]

        def v_group(ktile):
            """One 128-key tile of V for ALL 4 pairs (N=512)."""
            ps = pspj.tile([128, 512], F32, name="v_ps", tag="pspj")
            for dc in range(DC):
                nc.tensor.matmul(ps, xT_sb[:, dc, ktile * 128:(ktile + 1) * 128],
                                 wv_sb[:, dc, :],
                                 start=(dc == 0), stop=(dc == DC - 1))
            nc.vector.tensor_copy(v_sb[:, ktile, :, 0:64], ps)

        def attention(p, qt, work, ret_raw=False):
            """kt loop; pops one thunk from `work` per key tile. Returns
            deferred normalization closures (or, with ret_raw, the raw
            accumulator/broadcast tiles for the caller to normalize)."""
            q0 = qt * 512
            qt_sb = qk_tiles[(p, "q")]
            kt_sb = qk_tiles[(p, "k")]
            av = [psav.tile([65, 512], F32, name=f"av{hh}", tag="psav")
                  for hh in range(2)]

            def scores(kt):
                ps = pssc.tile([128, 1024], F32, name="sc_ps", tag="pssc")
                for hh in range(2):
                    hb = hh * 64
                    nc.tensor.matmul(
                        ps[:, hh * 512:(hh + 1) * 512],
                        kt_sb[hb:hb + 64, kt * 128:(kt + 1) * 128],
                        qt_sb[hb:hb + 64, q0:q0 + 512],
                        start=True, stop=True)
                return ps

            # scores are software-pipelined one key tile ahead so prep
            # thunks and attn@V never sit between scores(kt+1) and the exp
            # that needs it
            ps_next = scores(0)
            for kt in range(NKT):
                es = espool.tile([128, 1024], BF16, name="es", tag="es")
                nc.scalar.activation(es, ps_next, EXP, scale=0.125)
                if kt + 1 < NKT:
                    ps_next = scores(kt + 1)
                if work:
                    work.pop(0)()
                for hh in range(2):
                    nc.tensor.matmul(
                        av[hh], v_sb[:, kt, 2 * p + hh, :],
                        es[:, hh * 512:(hh + 1) * 512],
                        start=(kt == 0), stop=(kt == NKT - 1),
                        skip_group_check=True)
            # prompt part (DVE only, no DMA dependencies): free the PSUM
            # accumulators, take the reciprocal of the denominator row,
            # then broadcast it with a DMA. The multiplies run on the Pool
            # engine a few key tiles later so nothing in the DVE's in-order
            # queue ever waits on the DMA round trip.
            fin = []
            for hh in range(2):
                av_sb = small.tile([65, 512], F32, name="av_sb", tag="av_sb")
                nc.vector.tensor_copy(av_sb, av[hh])
                bc = small.tile([64, 512], F32, name="bc", tag="bc")
                sr = av_sb[64:65, :]
                rep = bass.AP(tensor=sr.tensor, offset=sr.offset,
                              ap=[sr.ap[0], [0, 64], [1, 512]])
                nc.gpsimd.dma_start(out=bc.unsqueeze(1), in_=rep)
                if ret_raw:
                    fin.append((av_sb, bc))
                    continue

                def finish(hh=hh, av_sb=av_sb, bc=bc):
                    rec = small.tile([64, 512], F32, name="rec", tag="rec")
                    nc.vector.reciprocal_approx_fast(out=rec, in_=bc)
                    if hh == 0:
                        nc.vector.tensor_mul(attnT[p][0:64, q0:q0 + 512],
                                             av_sb[0:64, :], rec)
                    else:
                        # engines can't shift partitions; route via DMA
                        tmp = small.tile([64, 512], BF16, name="tmp",
                                         tag="tmp")
                        nc.vector.tensor_mul(tmp, av_sb[0:64, :], rec)
                        nc.gpsimd.dma_start(
                            out=attnT[p][64:128, q0:q0 + 512], in_=tmp)
                fin.append(finish)
            return fin

        def outproj_half(qc, nt):
            o_ps = pspj.tile([128, 512], F32, name=f"o_ps{nt}", tag="pspj")
            for pp in range(NPAIRS):
                nc.tensor.matmul(
                    o_ps, attnT[pp][:, qc * 128:(qc + 1) * 128],
                    wo_sb[:, pp, nt * 512:(nt + 1) * 512],
                    start=(pp == 0), stop=(pp == NPAIRS - 1),
                    skip_group_check=True)
            o_sb = osb.tile([128, 512], BF16, name="o_sb", tag="o_sb")
            nc.vector.tensor_copy(o_sb, o_ps)
            nc.sync.dma_start(
                out=out[qc * 128:(qc + 1) * 128, nt * 512:(nt + 1) * 512],
                in_=o_sb)

        # --- main flow ----------------------------------------------------
        # pair 0 prep: K fully (kt loop needs it all), Q block 0, then all
        # of V while the input DMAs stream in; later projections and the
        # out-projection ride the per-block work queues
        for nt in range(4):
            proj_group(0, "k", nt)
        proj_group(0, "q", 0)
        for ktile in range(3):
            v_group(ktile)

        deferred = []
        # prep items (absolute target pair) consumed a few per block, in
        # deadline order: pair p's blocks run its own remaining Q blocks
        # (p0 only) then the next pair's projections K0..K3, Q0..Q3
        halves_cache = {}

        def get_half(tp, mat, nt, h):
            key = (tp, mat, nt)
            if key not in halves_cache:
                halves_cache[key] = proj_halves(tp, mat, nt)
            return halves_cache[key][h]

        def pair_prep(p):
            items = []
            if p == 0:
                for nt2 in range(1, 4):
                    items += [(0, "q", nt2, h) for h in range(2)]
            if p + 1 < NPAIRS:
                for nt2 in range(4):
                    items += [(p + 1, "k", nt2, h) for h in range(2)]
                for nt2 in range(4):
                    items += [(p + 1, "q", nt2, h) for h in range(2)]
            return items

        queue = []
        fine = {}
        for p in range(NPAIRS):
            queue = queue + pair_prep(p)
            for qt in range(NQT):
                last = (p == NPAIRS - 1 and qt == NQT - 1)
                work = []
                if p == NPAIRS - 1:
                    work += deferred
                    deferred = []
                if p == 0 and qt == 0:
                    work += [lambda kt=k: v_group(kt + 3)
                             for k in range(NKT - 3)]
                    cap = max(0, 16 - len(work) - 1)
                else:
                    cap = 5
                taken, queue = queue[:cap], queue[cap:]
                work += [get_half(*item) for item in taken]
                if p < NPAIRS - 1:
                    work += deferred
                    deferred = []
                else:
                    # out-projection of the previous query block last: its
                    # matmuls wait on the attnT writes issued at slots 0-1,
                    # so the prep in between hides the DMA round trip
                    if qt > 0:
                        while len(work) < 8:
                            work.append(lambda: None)
                        for qc in range((qt - 1) * 4, qt * 4):
                            work += [lambda qc=qc: outproj_half(qc, 0),
                                     lambda qc=qc: outproj_half(qc, 1)]
                deferred = attention(p, qt, work, ret_raw=last)
        # final block: per-column-block normalize pipelined with the
        # out-projection so the tail isn't one long serial chain
        av_bc = deferred
        recs = []
        for hh in range(2):
            av_sb, bc = av_bc[hh]
            rec = small.tile([64, 512], F32, name="rec", tag="rec")
            nc.vector.reciprocal_approx_fast(out=rec, in_=bc)
            recs.append(rec)
        for qc in range(12, 16):
            csl = slice(qc * 128 - 1536, qc * 128 - 1536 + 128)
            asl = slice(qc * 128, (qc + 1) * 128)
            nc.vector.tensor_mul(attnT[3][0:64, asl],
                                 av_bc[0][0][0:64, csl], recs[0][:, csl])
            tmp = small.tile([64, 128], BF16, name="tmpf", tag="tmpf")
            nc.vector.tensor_mul(tmp, av_bc[1][0][0:64, csl],
                                 recs[1][:, csl])
            nc.gpsimd.dma_start(out=attnT[3][64:128, asl], in_=tmp)
            outproj_half(qc, 0)
            outproj_half(qc, 1)
    nc.compile()
    return nc


def _get_compiled():
    global _COMPILED
    if _COMPILED is None:
        _COMPILED = _build()
    return _COMPILED


def make_in_maps(**inputs):
    import ml_dtypes
    bf16 = ml_dtypes.bfloat16
    x = np.asarray(inputs["inputs"], np.float32)
    # all tensors pre-arranged to the on-chip [partition, ...] layout so
    # every DMA is one large contiguous descriptor per partition
    xTb = [np.ascontiguousarray(
               x[b].T.reshape(DC, 128, S).transpose(1, 0, 2)).astype(bf16)
           for b in range(B)]
    gslice = {}
    for nm in ("Wq", "Wk", "Wv", "Wo", "bq", "bk"):
        a = np.asarray(inputs[nm], np.float32)
        for g in range(2):
            sl = slice(g * GCOLS, (g + 1) * GCOLS)
            if nm == "Wo":
                w = a[sl, :].reshape(NPAIRS, 128, D).transpose(1, 0, 2)
                gslice[(nm, g)] = np.ascontiguousarray(w).astype(bf16)
            elif nm.startswith("W"):
                w = a[:, sl].reshape(DC, 128, GCOLS).transpose(1, 0, 2)
                gslice[(nm, g)] = np.ascontiguousarray(w).astype(bf16)
            else:
                # pre-shaped [128, NPAIRS]: partition r = within-pair dim,
                # col p = pair index (avoids a 512-descriptor gather DMA)
                gslice[(nm, g)] = np.ascontiguousarray(
                    a[sl].reshape(NPAIRS, 128).T)
    in_maps = []
    for c in range(NCORES):
        g, b = c // B, c % B
        in_maps.append({
            "xT": xTb[b],
            "wq": gslice[("Wq", g)], "wk": gslice[("Wk", g)],
            "wv": gslice[("Wv", g)], "wo": gslice[("Wo", g)],
            "bq": gslice[("bq", g)], "bk": gslice[("bk", g)],
        })
    return in_maps


def combine(results, bo, bv, Wo):
    out = np.empty((B, S, D), np.float32)
    bo = np.asarray(bo, np.float32)
    bv = np.asarray(bv, np.float32)
    Wo = np.asarray(Wo, np.float32)
    const_row = bo + bv @ Wo
    for b in range(B):
        out[b] = (np.asarray(results[b]["out"], np.float32) +
                  np.asarray(results[B + b]["out"], np.float32) + const_row)
    return out


def kernel(**inputs):
    from concourse import bass_utils
    nc = _get_compiled()
    in_maps = make_in_maps(**inputs)
    res = bass_utils.run_bass_kernel_spmd(
        nc, in_maps, core_ids=list(range(NCORES)))
    return combine(res.results, inputs["bo"], inputs["bv"], inputs["Wo"])
